# revision 21
# baseline (speedup 1.0000x reference)
"""Trainium2 Bass kernel for nn_MPCActor (MLP -> condensed-QP LQR solve).

Math: the reference's Riccati sweep solves a finite-horizon LQR with
time-invariant diagonal cost diag(q), linear cost p, shared dynamics (A, B).
Condensing states out: u5 = -pu/qu elementwise, and U = [u1..u4] solves the
16x16 SPD system H U = r with
    H = Du + G^T diag(qx-bar) G      (linear in q via constant P_H)
    r = -G^T(qx-bar * (Mc x1)) - (SG^T px + pu-bar)
G, Mc, P_H, SG are constants built from A, B on the host.  Host also
pre-transposes obs ([22, bc]) and pre-computes cT = (Mc x1)^T ([48, bc]),
so the device does no transposes at all.

On device (per core, bc = 8192 batch):
  - L2 phase (features on partitions, batch on free): MLP on TensorE,
    relu/sigmoid evacs on ACT; per-128-block matmuls against constant maps
    emit [H | r | qu pu] flat (a-major: entry 16a+k = H[a,k], 256+k = r[k]);
    the ACT evac scatters each 128-block's 280 entries into the solve
    layout hs[p, entry, i] (i = i-slot, INNERMOST, 32 wide).
  - L1 phase: batched LDL^T on VectorE with the i-slot dim innermost, so
    every tensor_tensor op is contiguous fp16 step-1 -> DVE 2x packed mode
    (the old layout ran the dominant mul at 1x via a stride-0 inner
    broadcast).  Columns are scaled by 1/d in place below the diagonal,
    making the back-substitution rd-free; the augmented r row doubles as
    the forward-solve for free.
  - 2 solve groups of 32 i-slots; group g+1's chunk phase is emitted
    interleaved into group g's solve so PE/ACT overlap the long DVE chain.
Sharding: pure data parallel over batch across 8 cores.
"""
import sys
import numpy as np

for _p in ("/opt/trn_rl_repo",):
    if _p not in sys.path:
        sys.path.append(_p)

import concourse.bass as bass
import concourse.mybir as mybir
import concourse.tile as tile
from concourse import bacc
from concourse.bass_utils import run_bass_kernel_spmd

S, C, OBS, T, B, HID = 12, 4, 22, 5, 65536, 512
N = S + C
nU = (T - 1) * C   # 16
nX = (T - 1) * S   # 48
NCORES = 8
BC = B // NCORES   # 8192 per core
NB = 512           # chunk batch (free dim of L2 phase)
NCHUNK = BC // NB  # 16
NI = BC // 128     # 64 i-slots
NGRP = 2
GI = NI // NGRP    # 32 i per solve group
GCH = NCHUNK // NGRP  # 8 chunks per group
NEH = nU * (nU + 1)   # 272 = a-major [H rows a=0..15 | r row a=16] x 16
NE = NEH + 2 * C      # 280: + qu(4) + pu(4)
f32 = mybir.dt.float32
f32r = mybir.dt.float32r
f16 = mybir.dt.float16
bf16 = mybir.dt.bfloat16
AF = mybir.ActivationFunctionType
OP = mybir.AluOpType
AX = mybir.AxisListType


def make_consts(A, Bm):
    A = np.asarray(A, np.float64)
    Bm = np.asarray(Bm, np.float64)
    Apow = [np.eye(S)]
    for _ in range(T - 1):
        Apow.append(Apow[-1] @ A)
    G = np.zeros((nX, nU))
    Mc = np.zeros((nX, S))
    for i in range(1, T):
        Mc[(i - 1) * S:i * S] = Apow[i]
        for j in range(1, i + 1):
            G[(i - 1) * S:i * S, (j - 1) * C:j * C] = Apow[i - j] @ Bm
    Gr = G.reshape(T - 1, S, nU)
    P_H = np.einsum("tsi,tsj->ijs", Gr, Gr)
    SG = np.einsum("tsi->si", Gr)
    # y-layout [32, 280]: col 16a+k = H[a,k] (a-major), col 256+k = linear
    # part of r[k], cols 272:280 = qu, pu selectors.
    MH_full = np.zeros((2 * N, nU * nU))
    for i in range(nU):
        for j in range(nU):
            MH_full[:S, i * nU + j] = P_H[i, j]
        MH_full[S + i % C, i * nU + i] += 1.0
    Mrlin = np.zeros((2 * N, nU))
    Mrlin[N:N + S, :] = -SG
    for i in range(nU):
        Mrlin[N + S + i % C, i] += -1.0
    Mqbar = np.zeros((2 * N, nX))
    for t in range(T - 1):
        for s in range(S):
            Mqbar[s, t * S + s] = 1.0
    SEL = np.zeros((2 * N, 2 * C))
    for c in range(C):
        SEL[S + c, c] = 1.0          # qu
        SEL[N + S + c, C + c] = 1.0  # pu
    MHA = np.concatenate([MH_full, Mrlin, SEL], axis=1)   # [32, 280]
    z = np.float32
    return dict(MHF=MHA.astype(z), Mqbar=Mqbar.astype(z),
                Gneg=(-G).astype(z), Mc=Mc.astype(z))


def build(bc=BC, repeat=1):
    assert bc == BC
    nc = bacc.Bacc("TRN2", target_bir_lowering=False, debug=False)

    obsT_d = nc.declare_dram_parameter("obsT", [54, bc], bf16, isOutput=False)
    # packed: rows 0:48 = c^T for even 256-halves, 64:112 for odd halves
    cT_d = nc.declare_dram_parameter("cT", [128, bc // 2], bf16, isOutput=False)
    W1_d = nc.declare_dram_parameter("W1", [2 * 32 - 10, HID], bf16, isOutput=False)
    b1_d = nc.declare_dram_parameter("b1", [HID], f32, isOutput=False)
    W2_d = nc.declare_dram_parameter("W2", [HID, HID], bf16, isOutput=False)
    b2_d = nc.declare_dram_parameter("b2", [HID], f32, isOutput=False)
    W3_d = nc.declare_dram_parameter("W3", [HID, 4 * N], bf16, isOutput=False)
    b3_d = nc.declare_dram_parameter("b3", [128], f32, isOutput=False)
    MHF_d = nc.declare_dram_parameter("MHF", [2 * N, NE], bf16, isOutput=False)
    MQB_d = nc.declare_dram_parameter("Mqbar", [2 * N, nX], bf16, isOutput=False)
    GNEG_d = nc.declare_dram_parameter("Gneg", [nX, nU], bf16, isOutput=False)
    # output: per group 20 rows (16 = U via LDL solve, 4 = u5), i innermost
    u_d = nc.declare_dram_parameter("u", [128, NGRP * (nU + C) * GI], f16,
                                    isOutput=True)

    with tile.TileContext(nc) as tc:
        with tc.tile_pool(name="const", bufs=1) as cp, \
             tc.tile_pool(name="inp", bufs=4) as ip, \
             tc.tile_pool(name="work", bufs=3) as wp, \
             tc.tile_pool(name="hs", bufs=3) as hp, \
             tc.tile_pool(name="slv", bufs=2) as sp, \
             tc.tile_pool(name="psmm", bufs=2, space="PSUM") as pmm, \
             tc.tile_pool(name="pssm", bufs=1, space="PSUM") as psm, \
             tc.tile_pool(name="psh", bufs=1, space="PSUM") as phh:

            # ---- constants ----
            w1sb = cp.tile([54, HID], bf16, tag="w1")
            nc.sync.dma_start(out=w1sb, in_=W1_d.ap())
            w2sb = []
            for k in range(4):
                t_ = cp.tile([128, HID], bf16, tag=f"w2_{k}")
                nc.sync.dma_start(out=t_, in_=W2_d.ap()[128 * k:128 * (k + 1), :])
                w2sb.append(t_)
            w3sb = []
            for k in range(4):
                t_ = cp.tile([128, 4 * N], bf16, tag=f"w3_{k}")
                nc.sync.dma_start(out=t_, in_=W3_d.ap()[128 * k:128 * (k + 1), :])
                w3sb.append(t_)
            b1sb = cp.tile([128, 4], f32, tag="b1")
            nc.sync.dma_start(out=b1sb, in_=b1_d.ap().rearrange("(m p) -> p m", p=128))
            b2sb = cp.tile([128, 4], f32, tag="b2")
            nc.sync.dma_start(out=b2sb, in_=b2_d.ap().rearrange("(m p) -> p m", p=128))
            b3sb = cp.tile([128, 1], f32, tag="b3")
            nc.sync.dma_start(out=b3sb, in_=b3_d.ap().rearrange("(m o) -> m o", o=1))
            mhf = cp.tile([128, NE], bf16, tag="mhf")
            for q in range(4):
                nc.sync.dma_start(out=mhf[32 * q:32 * (q + 1), :],
                                  in_=MHF_d.ap())
            mqb = cp.tile([96, nX], bf16, tag="mqb")
            nc.sync.dma_start(out=mqb[0:2 * N, :], in_=MQB_d.ap())
            nc.sync.dma_start(out=mqb[64:64 + 2 * N, :], in_=MQB_d.ap())
            gneg = cp.tile([112, nU], bf16, tag="gneg")
            nc.sync.dma_start(out=gneg[0:nX, :], in_=GNEG_d.ap())
            nc.sync.dma_start(out=gneg[64:64 + nX, :], in_=GNEG_d.ap())

            hs_tiles = {}

            def chunk_phase(rep, pair):
                # Two chunks per phase: every stationary weight feeds >=2
                # back-to-back matmuls, so the PE drain pipelines (isolated
                # MM ~379ns vs pipelined ~216ns), and the PSUM evacs fuse.
                ch0 = 2 * pair
                g = ch0 // GCH
                NH = NB // 2
                if (rep, g) not in hs_tiles:
                    # 288 = 18*16 flat: rows a=0..16 = augmented [H|r] matrix
                    # (entry 16a+k), row 17 = [qu(4) pu(4) pad(8)]; i innermost
                    hs_tiles[(rep, g)] = hp.tile([128, 288, GI], f16, tag="hs",
                                                 name=f"hs_{rep}_{g}")
                hs = hs_tiles[(rep, g)]

                obs_c = ip.tile([54, 2 * NB], bf16, tag="obs_c")
                nc.sync.dma_start(out=obs_c,
                                  in_=obsT_d.ap()[:, ch0 * NB:(ch0 + 2) * NB])
                cT_c = ip.tile([128, NB], bf16, tag="cT_c")
                nc.sync.dma_start(out=cT_c,
                                  in_=cT_d.ap()[:, ch0 * NH:(ch0 + 2) * NH])
                cT_v = cT_c.rearrange("p (s b) -> p s b", s=2)

                # layer 1 (row-packed pairs: strips at partitions 0 and 32)
                h1sb = []
                for pr in range(2):
                    pss = []
                    for q in range(2):
                        mc = 2 * pr + q
                        ps = pmm.tile([128, 2, NB], f32, tag="mm")
                        for s in range(2):
                            nc.tensor.matmul(
                                out=ps[:, s, :],
                                lhsT=w1sb[32 * q:32 * q + OBS,
                                          128 * mc:128 * (mc + 1)],
                                rhs=obs_c[32 * q:32 * q + OBS,
                                          NB * s:NB * (s + 1)],
                                start=True, stop=True, tile_position=(32 * q, 0))
                        pss.append(ps)
                    for q in range(2):
                        mc = 2 * pr + q
                        hsb = wp.tile([128, 2, NB], bf16, tag=f"h1_{mc}")
                        nc.scalar.activation(out=hsb, in_=pss[q], func=AF.Relu,
                                             bias=b1sb[:, mc:mc + 1], scale=1.0)
                        h1sb.append(hsb)
                # layer 2
                h2sb = []
                for mc in range(4):
                    ps = pmm.tile([128, 2, NB], f32, tag="mm")
                    for kc in range(4):
                        for s in range(2):
                            nc.tensor.matmul(
                                out=ps[:, s, :],
                                lhsT=w2sb[kc][:, 128 * mc:128 * (mc + 1)],
                                rhs=h1sb[kc][:, s, :],
                                start=(kc == 0), stop=(kc == 3))
                    hsb = wp.tile([128, 2, NB], bf16, tag=f"h2_{mc}")
                    nc.scalar.activation(out=hsb, in_=ps, func=AF.Relu,
                                         bias=b2sb[:, mc:mc + 1], scale=1.0)
                    h2sb.append(hsb)
                # layer 3 + sigmoid, packed: batch half h -> partitions 64h.
                # One accumulation group per PSUM bank at a time (start=True
                # clears has_written bank-wide): bank = s, h serialized with
                # its own column region.
                ps_y = psm.tile([128, 2, NB], f32, tag="sm")
                for h in range(2):
                    for kc in range(4):
                        for s in range(2):
                            nc.tensor.matmul(
                                out=ps_y[64 * h:64 * h + 4 * N, s,
                                         NH * h:NH * (h + 1)],
                                lhsT=w3sb[kc],
                                rhs=h2sb[kc][:, s, NH * h:NH * (h + 1)],
                                start=(kc == 0), stop=(kc == 3),
                                tile_position=(0, 64 * h))
                ysb = wp.tile([128, 2, NH], bf16, tag="ysb")
                for h in range(2):
                    nc.scalar.activation(
                        out=ysb[64 * h:64 * h + 64, :, :],
                        in_=ps_y[64 * h:64 * h + 64, :, NH * h:NH * (h + 1)],
                        func=AF.Sigmoid, bias=b3sb[64 * h:64 * h + 64, 0:1],
                        scale=1.0)

                # qxbar = Mqbar^T y ; prod = qxbar * c   (c precomputed on host)
                ps_qb = psm.tile([128, 2, NH], f32, tag="sm")
                for h in range(2):
                    for s in range(2):
                        nc.tensor.matmul(out=ps_qb[64 * h:64 * h + nX, s, :],
                                         lhsT=mqb[64 * h:64 * h + 2 * N, :],
                                         rhs=ysb[64 * h:64 * h + 2 * N, s, :],
                                         start=True, stop=True,
                                         tile_position=(64 * h, 64 * h))
                qb_sb = wp.tile([128, 2, NH], bf16, tag="qb_sb")
                nc.scalar.copy(out=qb_sb, in_=ps_qb)
                prod = wp.tile([128, 2, NH], bf16, tag="prod")
                nc.vector.tensor_mul(out=prod, in0=cT_v, in1=qb_sb)

                # [H | r | qu pu] flat a-major via per-block matmuls, then
                # ACT-scatter both q's 280 entries into hs[:, :, i] columns
                for s in range(2):
                    ch = ch0 + s
                    il0 = 4 * (ch - g * GCH)
                    for pr in range(2):
                        ps_h = phh.tile([128, 2, NB], f32, tag="ps_h")
                        for q in range(2):
                            base = 64 * pr + 32 * q
                            blk = slice(128 * q, 128 * (q + 1))
                            nc.tensor.matmul(out=ps_h[:, q, 0:NE],
                                             lhsT=ysb[base:base + 2 * N, s, blk],
                                             rhs=mhf[base:base + 2 * N, :],
                                             start=True, stop=False,
                                             tile_position=(base, 0))
                        for q in range(2):
                            blk = slice(128 * q, 128 * (q + 1))
                            nc.tensor.matmul(
                                out=ps_h[:, q, nU * nU:nU * nU + nU],
                                lhsT=prod[64 * pr:64 * pr + nX, s, blk],
                                rhs=gneg[64 * pr:64 * pr + nX, :],
                                start=False, stop=True,
                                tile_position=(64 * pr, 0))
                        # scatter: out [p, e(280) stride GI, q(2) stride 1],
                        # src [p, e stride 1, q stride NB]
                        i0 = il0 + 2 * pr
                        nc.scalar.copy(
                            out=hs[:, 0:NE, i0:i0 + 2],
                            in_=ps_h[:, :, 0:NE].transpose([0, 2, 1]))

            def solve_group(rep, g, hook):
                import contextlib
                with nc.allow_low_precision(
                        reason="fp16 LDL validated against 2e-2 tolerance"):
                    _solve_group(rep, g, hook)

            def _solve_group(rep, g, hook):
                n = GI
                hs = hs_tiles.pop((rep, g))
                M = hs[:, 0:NEH, :].rearrange("p (a k) i -> p a k i", k=nU)
                diag = hs[:, 0:nU * (nU + 1):nU + 1, :]   # [p, 16, n] d_k
                fk = sp.tile([128, nU, n], f16, tag="fk")
                tmp = sp.tile([128, 72, n], f16, tag="tmp")
                rd = sp.tile([128, nU, n], f16, tag="rd")
                tb = sp.tile([128, nU, n], f16, tag="tb")
                zz = sp.tile([128, nU + C, n], f16, tag="zz")
                rq = sp.tile([128, C, n], f16, tag="rq")

                def tree(view, w):
                    # pairwise-halve view[..., 0:w, :] along its 2nd-to-last
                    # free dim until width 1 (result in slot 0)
                    while w > 1:
                        h = w // 2
                        nc.vector.tensor_add(out=view[:, :, 0:h, :],
                                             in0=view[:, :, 0:h, :],
                                             in1=view[:, :, w - h:w, :])
                        w -= h

                # in-place LDL: after step j, M[a, j] (a > j) holds l_j[a]
                # (scaled by 1/d_j), M[j, j] holds d_j; row 16 is the r row
                # (becomes w = D^-1 L^-1 r for free).
                for j in range(nU):
                    m = nU + 1 - j
                    if j > 0:
                        # fk[k] = c_k[j] = l_k[j] * d_k  (k < j)
                        nc.vector.tensor_mul(out=fk[:, 0:j, :],
                                             in0=M[:, j, 0:j, :],
                                             in1=diag[:, 0:j, :])
                        tr = tmp[:, 0:m * j, :].rearrange(
                            "p (a k) i -> p a k i", k=j)
                        nc.vector.tensor_mul(
                            out=tr, in0=M[:, j:nU + 1, 0:j, :],
                            in1=fk[:, 0:j, :].unsqueeze(1).broadcast_to(
                                [128, m, j, n]))
                        tree(tr, j)
                        nc.vector.tensor_sub(out=M[:, j:nU + 1, j, :],
                                             in0=M[:, j:nU + 1, j, :],
                                             in1=tmp[:, 0:m * j:j, :])
                    nc.vector.reciprocal(out=rd[:, j, :],
                                         in_=M[:, j, j, :])
                    nc.vector.tensor_mul(
                        out=M[:, j + 1:nU + 1, j, :],
                        in0=M[:, j + 1:nU + 1, j, :],
                        in1=rd[:, j, :].unsqueeze(1).broadcast_to(
                            [128, m - 1, n]))
                    # HAM keep-warm: a ~30ns weight-load gated on this step's
                    # rd write, so TensorE never idles >3.4us mid-solve and
                    # the clock gate stays at 8/8 (2.4 GHz) for the next
                    # chunk.  (Harmless: every real matmul self-loads.)
                    nc.tensor.ldweights(weights=rd[0:32, j, 0:16])
                    hook(j)

                # back-substitution: z[j] = w[j] - sum_{a>j} l_j[a] z[a],
                # w = scaled r row = M[16, :, :]
                nc.vector.tensor_copy(out=zz[:, nU - 1, :],
                                      in_=M[:, nU, nU - 1, :])
                for j in range(nU - 2, -1, -1):
                    m = nU - 1 - j
                    nc.vector.tensor_mul(out=tb[:, 0:m, :],
                                         in0=M[:, j + 1:nU, j, :],
                                         in1=zz[:, j + 1:nU, :])
                    w = m
                    while w > 1:
                        h = w // 2
                        nc.vector.tensor_add(out=tb[:, 0:h, :],
                                             in0=tb[:, 0:h, :],
                                             in1=tb[:, w - h:w, :])
                        w -= h
                    nc.vector.tensor_sub(out=zz[:, j, :],
                                         in0=M[:, nU, j, :],
                                         in1=tb[:, 0, :])
                # u5 = -pu/qu
                nc.vector.reciprocal(out=rq, in_=hs[:, NEH:NEH + C, :])
                nc.vector.scalar_tensor_tensor(
                    out=zz[:, nU:nU + C, :], in0=rq, scalar=-1.0,
                    in1=hs[:, NEH + C:NEH + 2 * C, :],
                    op0=OP.mult, op1=OP.mult)
                w0 = g * (nU + C) * GI
                nc.sync.dma_start(out=u_d.ap()[:, w0:w0 + (nU + C) * GI],
                                  in_=zz)

            # ---- flat schedule: pairs emitted one group ahead of solves ----
            NPG = GCH // 2   # chunk-pairs per group
            pending = [(rep, p) for rep in range(repeat)
                       for p in range(NCHUNK // 2)]
            pos = 0

            def emit_next_chunk():
                nonlocal pos
                if pos < len(pending):
                    chunk_phase(*pending[pos])
                    pos += 1

            for _ in range(NPG):
                emit_next_chunk()
            for rep in range(repeat):
                for g in range(NGRP):
                    done = [0]

                    def hook(j, done=done):
                        want = ((j + 1) * NPG) // nU
                        while done[0] < want:
                            emit_next_chunk()
                            done[0] += 1
                    solve_group(rep, g, hook)

    nc.compile()
    return nc


_NC_CACHE = {}


def _get_nc(bc):
    if bc not in _NC_CACHE:
        _NC_CACHE[bc] = build(bc)
    return _NC_CACHE[bc]


def make_in_maps(obs, x_init, W1, b1, W2, b2, W3, b3, A, Bm):
    """Per-core input dicts (host does transpose + Mc x1)."""
    import ml_dtypes
    bf = ml_dtypes.bfloat16
    obs = np.ascontiguousarray(obs, np.float32)
    x_init = np.ascontiguousarray(x_init, np.float32)
    cst = make_consts(A, Bm)
    Mc = cst.pop("Mc")
    W1r = np.zeros((54, HID), np.float32)
    W1r[0:OBS] = W1
    W1r[32:32 + OBS] = W1
    W3d = np.concatenate([W3, W3], axis=1)
    b3d = np.concatenate([b3, b3, b3, b3])
    shared = dict(W1=np.ascontiguousarray(W1r, bf),
                  b1=np.ascontiguousarray(b1, np.float32),
                  W2=np.ascontiguousarray(W2, bf),
                  b2=np.ascontiguousarray(b2, np.float32),
                  W3=np.ascontiguousarray(W3d, bf),
                  b3=np.ascontiguousarray(b3d, np.float32),
                  MHF=cst["MHF"].astype(bf), Mqbar=cst["Mqbar"].astype(bf),
                  Gneg=cst["Gneg"].astype(bf))
    cfull = x_init @ Mc.T  # [B, 48]
    in_maps = []
    for k in range(NCORES):
        sl = slice(k * BC, (k + 1) * BC)
        obsT2 = np.zeros((54, BC), np.float32)
        obsT2[0:OBS] = obs[sl].T
        obsT2[32:32 + OBS] = obs[sl].T
        # cT packed: rows 0:48 even 256-halves, rows 64:112 odd halves
        c3 = cfull[sl].T.reshape(nX, NCHUNK, 2, NB // 2)
        cT2 = np.zeros((128, BC // 2), np.float32)
        cT2[0:nX] = c3[:, :, 0, :].reshape(nX, -1)
        cT2[64:64 + nX] = c3[:, :, 1, :].reshape(nX, -1)
        in_maps.append(dict(obsT=np.ascontiguousarray(obsT2, bf),
                            cT=np.ascontiguousarray(cT2, bf),
                            **shared))
    return in_maps


def unshard_u(res_list):
    """[128, NGRP*20*GI] per core -> [T, B, C]."""
    out = np.empty((T, B, C), np.float32)
    for k in range(NCORES):
        u_dev = np.asarray(res_list[k]).astype(np.float32)
        u_dev = u_dev.reshape(128, NGRP, nU + C, GI)
        # batch element b = (g*GI + ii)*128 + p; row k<16 = u[k//4][b][k%4],
        # rows 16:20 = u[4]
        arr = u_dev.transpose(1, 3, 0, 2).reshape(BC, T, C)
        out[:, k * BC:(k + 1) * BC, :] = arr.transpose(1, 0, 2)
    return out


def kernel(obs, x_init, W1, b1, W2, b2, W3, b3, A, Bm):
    nc = _get_nc(BC)
    in_maps = make_in_maps(obs, x_init, W1, b1, W2, b2, W3, b3, A, Bm)
    res = run_bass_kernel_spmd(nc, in_maps, list(range(NCORES)))
    return unshard_u([res.results[k]["u"] for k in range(NCORES)])
